# revision 11
# baseline (speedup 1.0000x reference)
"""Trainium2 Bass kernel for nn_H_layer_85512798863503 (GNN message passing / GAT-style).

Strategy (self-contained; shapes hardcoded):
  - Shard edges across 8 cores by OWNER OF DST NODE (6250 nodes/core) so all
    segment reductions (softmax max/sum, weighted aggregation, er mean) are
    core-local -> no collectives.
  - Within a core, group edges by 64-node dst blocks (sorted by dst). Segment
    sums become PSUM-accumulated one-hot matmuls on TensorE.
  - Per-edge features come from a transpose-mode dma_gather of bf16 x rows
    (feature-major), then the three linear layers are applied PER EDGE by
    TensorE (weights folded on host: xs/h/s_src in one matmul; xd/s_dst/bl
    added by a one-hot "expansion" matmul against an SBUF-resident dst table).
  - Softmax max-subtraction is dropped: scores are O(1)-bounded so exp() can't
    overflow fp32, and the softmax value is mathematically identical.
"""
import sys
if "/opt/trn_rl_repo" not in sys.path:
    sys.path.insert(0, "/opt/trn_rl_repo")

import numpy as np
import ml_dtypes

F16 = np.float16
EXPSHIFT = -5.54  # exp(a+EXPSHIFT): keeps e in fp16 range; cancels in softmax ratio

N, E, DIN, HEAD, HD = 50000, 800000, 128, 4, 16
DOUT = HEAD * HD            # 64
NCORES = 8
NPC = N // NCORES           # 6250 nodes per core
NB = 64                     # dst nodes per block
NBLK = (NPC + NB - 1) // NB # 98
HALF = 25000                # int16 gather index limit workaround: two x views
NPAD = NBLK * NB            # 6272 padded nodes per core
XROWS = 50176               # 392*128 padded x rows
NEG = 0.01


def _blockdiag(w):
    m = np.zeros((DOUT, HEAD), np.float32)
    for h in range(HEAD):
        m[16 * h:16 * h + 16, h] = w
    return m


def _host_prep(x, src, dst, Ws, bs, Wd, bd, Wl, bl, Wa, ba):
    f32 = np.float32
    x = np.asarray(x, f32); src = np.asarray(src); dst = np.asarray(dst)

    # ---- weight folding ----
    WaS, WaD, WaE = Wa[0:16, 0], Wa[16:32, 0], Wa[32:48, 0]
    WaS_bd, WaD_bd = _blockdiag(WaS), _blockdiag(WaD)
    wsrc = np.concatenate([Ws, Wl @ WaS_bd, Wl], axis=1).astype(F16)        # [128,132]
    wnode = np.concatenate([Wl, Wd, Wl @ WaD_bd], axis=1).astype(f32)        # [128,132]
    bias_h = np.tile(bl.astype(f32)[None, :], (64, 1))                       # [64,64]
    bdst = np.concatenate([bs + bd, bl @ WaS_bd + bl @ WaD_bd + ba]).astype(f32)
    bias_dst = np.tile(bdst[None, :], (64, 1))                               # [64,68]
    wae_row = np.tile(WaE[np.arange(DOUT) % 16][None, :], (128, 1)).astype(F16)
    bl_bf = np.tile(bl.astype(F16)[None, :], (64, 1))                       # [64,64]

    x_pad = np.zeros((XROWS, DIN), f32)
    x_pad[:N] = x
    x_bf = x_pad.astype(F16)

    deg = np.bincount(dst, minlength=N).astype(f32)

    # ---- edge binning ----
    core_of = dst // NPC
    counts = np.zeros((NCORES, NBLK, 2), np.int64)
    per_core = []
    for c in range(NCORES):
        ei = np.nonzero(core_of == c)[0]
        dl = dst[ei] - c * NPC
        blk = dl // NB
        half = (src[ei] >= HALF).astype(np.int64)
        key = blk * 2 + half
        counts[c] = np.bincount(key, minlength=NBLK * 2).reshape(NBLK, 2)
        per_core.append((ei, dl, blk, half, key))

    cmax = counts.max(axis=0)                                   # [NBLK,2]
    caps = ((cmax + 127) // 128) * 128                          # [NBLK,2]
    capflat = caps.reshape(-1)
    offs = np.zeros(2 * NBLK + 1, np.int64)
    np.cumsum(capflat, out=offs[1:])
    STOT = int(offs[-1])
    IDXCOLS = STOT // 16

    shared = dict(
        xbf=x_bf, wsrc=wsrc, wnode=wnode, waer=wae_row,
        bhr=bias_h, bdr=bias_dst, blbf=bl_bf,
    )

    per_core_maps = []
    for c in range(NCORES):
        ei, dl, blk, half, key = per_core[c]
        order = np.argsort(key, kind="stable")
        ks = key[order]
        grp_start_per_edge = np.searchsorted(ks, ks)
        rank = np.arange(len(ks)) - grp_start_per_edge
        pos = offs[ks] + rank
        srcidx = np.zeros(STOT, np.int16)
        dstloc = np.full(STOT, -1, np.int16)
        s_sorted = src[ei][order]
        srcidx[pos] = (s_sorted - HALF * (s_sorted >= HALF)).astype(np.int16)
        dstloc[pos] = (dl[order] - blk[order] * NB).astype(np.int16)

        oh = np.zeros((STOT, NB), F16)
        valid = dstloc >= 0
        oh[np.nonzero(valid)[0], dstloc[valid].astype(np.int64)] = 1
        ohT = np.ascontiguousarray(oh.T)

        idxbuf = np.tile(srcidx.reshape(IDXCOLS, 16).T, (8, 1))  # [128, IDXCOLS]

        node_ids = c * NPC + np.arange(NPAD)
        degc = np.ones(NPAD, f32)
        in_range = node_ids < min((c + 1) * NPC, N)
        degc[in_range] = np.maximum(deg[node_ids[in_range]], 1.0)
        ivd = np.ascontiguousarray((1.0 / degc).reshape(NBLK, NB).T)  # [64, NBLK]

        xsl = np.ascontiguousarray(x_pad[c * NPC: c * NPC + NPAD].T)  # [128, NPAD]

        m = dict(shared)
        m.update(idxs=idxbuf, oh=oh, oht=ohT, ivd=ivd, xsl=xsl)
        per_core_maps.append(m)

    return caps, STOT, IDXCOLS, per_core_maps


def _build_program(caps, STOT, IDXCOLS):
    import concourse.bass as bass
    import concourse.mybir as mybir
    import concourse.tile as tile
    from concourse import bacc
    from contextlib import ExitStack

    dt = mybir.dt
    Alu = mybir.AluOpType
    Act = mybir.ActivationFunctionType

    nc = bacc.Bacc("TRN2", target_bir_lowering=False, debug=False,
                   num_devices=NCORES)

    xbf = nc.dram_tensor("xbf", [XROWS, DIN], dt.float16, kind="ExternalInput").ap()
    xsl = nc.dram_tensor("xsl", [DIN, NPAD], dt.float32, kind="ExternalInput").ap()
    wsrc = nc.dram_tensor("wsrc", [128, 132], dt.float16, kind="ExternalInput").ap()
    wnode = nc.dram_tensor("wnode", [128, 132], dt.float32, kind="ExternalInput").ap()
    waer = nc.dram_tensor("waer", [128, 64], dt.float16, kind="ExternalInput").ap()
    bhr = nc.dram_tensor("bhr", [64, 64], dt.float32, kind="ExternalInput").ap()
    bdr = nc.dram_tensor("bdr", [64, 68], dt.float32, kind="ExternalInput").ap()
    blbf = nc.dram_tensor("blbf", [64, 64], dt.float16, kind="ExternalInput").ap()
    idxs = nc.dram_tensor("idxs", [128, IDXCOLS], dt.int16, kind="ExternalInput").ap()
    ohd = nc.dram_tensor("oh", [STOT, NB], dt.float16, kind="ExternalInput").ap()
    ohtd = nc.dram_tensor("oht", [NB, STOT], dt.float16, kind="ExternalInput").ap()
    ivd = nc.dram_tensor("ivd", [NB, NBLK], dt.float32, kind="ExternalInput").ap()
    hout = nc.dram_tensor("hout", [NPAD, DOUT], dt.float32, kind="ExternalOutput").ap()
    esout = nc.dram_tensor("esout", [NPAD, 128], dt.float32, kind="ExternalOutput").ap()

    with tile.TileContext(nc) as tc:
        with ExitStack() as ctx:
            const = ctx.enter_context(tc.tile_pool(name="const", bufs=1))
            big = ctx.enter_context(tc.tile_pool(name="big", bufs=1))

            def cload(shape, dtyp, dram, tag):
                t = const.tile(shape, dtyp, tag=tag)
                nc.sync.dma_start(t[:], dram[:])
                return t

            wsrc_sb = cload([128, 132], dt.float16, wsrc, "wsrc")
            wnode_sb = cload([128, 132], dt.float32, wnode, "wnode")
            waer_sb = cload([128, 64], dt.float16, waer, "waer")
            bhr_sb = cload([64, 64], dt.float32, bhr, "bhr")
            bdr_sb = cload([64, 68], dt.float32, bdr, "bdr")
            blbf_sb = cload([64, 64], dt.float16, blbf, "blbf")
            ivd_sb = cload([NB, NBLK], dt.float32, ivd, "ivd")

            ebias = const.tile([128, 1], dt.float32)
            nc.vector.memset(ebias[:], EXPSHIFT)

            _regcache = {}

            def nreg(v):
                if v not in _regcache:
                    _regcache[v] = nc.gpsimd.to_reg(v)
                return _regcache[v]

            idx_sb = big.tile([128, IDXCOLS], dt.int16)
            nc.sync.dma_start(idx_sb[:], idxs[:])

            dstTab = big.tile([64, NBLK * 132], dt.float16)
            dstTab3 = dstTab[:].rearrange("p (t c) -> p t c", c=132)
            esb = big.tile([64, NBLK * 128], dt.float32)
            es3 = esb[:].rearrange("p (t c) -> p t c", c=128)
            nc.vector.memset(esb[:], 0.0)

            # constant bl columns of dstTab (cols 68:132 of each 132-block)
            for t in range(NBLK):
                nc.vector.tensor_copy(out=dstTab3[:, t, 68:132], in_=blbf_sb[:])

            # ---------------- P1: node phase ----------------
            with tc.tile_pool(name="p1x", bufs=3) as p1x, \
                 tc.tile_pool(name="p1ps", bufs=3, space="PSUM") as p1ps, \
                 tc.tile_pool(name="p1o", bufs=4) as p1o:
                for t in range(NBLK // 2):
                    xT = p1x.tile([128, 128], dt.float32, tag="xT")
                    nc.sync.dma_start(xT[:], xsl[:, t * 128:(t + 1) * 128])
                    for sub in range(2):
                        b = 2 * t + sub
                        ps = p1ps.tile([64, 132], dt.float32)
                        nc.tensor.matmul(ps[:], lhsT=xT[:, sub * 64:sub * 64 + 64],
                                         rhs=wnode_sb[:], start=True, stop=True)
                        ht = p1o.tile([64, DOUT], dt.float32)
                        nc.vector.tensor_tensor(out=ht[:], in0=ps[:, 0:64],
                                                in1=bhr_sb[:], op=Alu.add)
                        nc.sync.dma_start(hout[b * 64:(b + 1) * 64, :], ht[:])
                        nc.vector.tensor_tensor(out=dstTab3[:, b, 0:68],
                                                in0=ps[:, 64:132], in1=bdr_sb[:],
                                                op=Alu.add)

            # ---------------- P2: edge phase ----------------
            with tc.tile_pool(name="xg", bufs=3) as xgp, \
                 tc.tile_pool(name="ohp", bufs=3) as ohp, \
                 tc.tile_pool(name="ohtp", bufs=3) as ohtp, \
                 tc.tile_pool(name="vp", bufs=3) as vp, \
                 tc.tile_pool(name="scr", bufs=4) as scr, \
                 tc.tile_pool(name="psE", bufs=6, space="PSUM") as psEp, \
                 tc.tile_pool(name="psV", bufs=2, space="PSUM") as psVp:

                off = 0
                for b in range(NBLK):
                    Clo, Chi = int(caps[b, 0]), int(caps[b, 1])
                    Sb = Clo + Chi
                    if Sb == 0:
                        continue
                    nb2 = Sb // 128

                    xg = xgp.tile([128, Sb], dt.float16, tag="xg")
                    if Clo:
                        nc.gpsimd.dma_gather(
                            out_ap=xg[:, 0:Clo].rearrange("p (a b) -> p a b", a=1),
                            in_ap=xbf[0:HALF, :],
                            idxs_ap=idx_sb[:, off // 16: off // 16 + Clo // 16],
                            num_idxs=Clo, num_idxs_reg=nreg(Clo), elem_size=DIN,
                            transpose=True)
                    if Chi:
                        nc.gpsimd.dma_gather(
                            out_ap=xg[:, Clo:Sb].rearrange("p (a b) -> p a b", a=1),
                            in_ap=xbf[HALF:2 * HALF, :],
                            idxs_ap=idx_sb[:, (off + Clo) // 16: (off + Sb) // 16],
                            num_idxs=Chi, num_idxs_reg=nreg(Chi), elem_size=DIN,
                            transpose=True)

                    oh_sb = ohp.tile([128, nb2 * NB], dt.float16, tag="oh")
                    oh3 = oh_sb[:].rearrange("p (t c) -> p t c", c=NB)
                    nc.sync.dma_start(
                        oh3, ohd[off:off + Sb, :].rearrange("(t p) c -> p t c", p=128))
                    oht_sb = ohtp.tile([NB, Sb], dt.float16, tag="oht")
                    nc.sync.dma_start(oht_sb[:], ohtd[:, off:off + Sb])

                    V = vp.tile([128, nb2 * 132], dt.float16, tag="V")
                    V3 = V[:].rearrange("p (t c) -> p t c", c=132)
                    psV = psVp.tile([NB, 132], dt.float32)
                    dtab = dstTab3[:, b, :]

                    ngrp = (nb2 + 2) // 3
                    for g in range(ngrp):
                        nt = min(3, nb2 - 3 * g)
                        psE = psEp.tile([128, 396], dt.float32)
                        psE3 = psE[:].rearrange("p (t c) -> p t c", c=132)
                        for tt in range(nt):
                            t = 3 * g + tt
                            nc.tensor.matmul(psE3[:, tt, :],
                                             lhsT=xg[:, t * 128:(t + 1) * 128],
                                             rhs=wsrc_sb[:], start=True, stop=False)
                            nc.tensor.matmul(psE3[:, tt, :],
                                             lhsT=oht_sb[:, t * 128:(t + 1) * 128],
                                             rhs=dtab, start=False, stop=True)
                        # er = tanh(xs+xd') -> V[:, :, 64:128] (bf16)
                        nc.scalar.activation(out=V3[:, 3 * g:3 * g + nt, 64:128],
                                             in_=psE3[:, 0:nt, 0:64], func=Act.Tanh)
                        # s_er = sum over hd of er*WaE
                        tmp = scr.tile([128, 3 * 64], dt.float32, tag="tmp")
                        t3 = tmp[:].rearrange("p (t c) -> p t c", c=64)
                        nc.vector.tensor_tensor(
                            out=t3[:, 0:nt, :],
                            in0=V3[:, 3 * g:3 * g + nt, 64:128],
                            in1=waer_sb[:].rearrange("p c -> p () c")
                                .to_broadcast([128, nt, 64]),
                            op=Alu.mult)
                        ser = scr.tile([128, 3 * 4], dt.float32, tag="ser")
                        ser3 = ser[:].rearrange("p (t c) -> p t c", c=4)
                        nc.vector.tensor_reduce(
                            out=ser3[:, 0:nt, :],
                            in_=t3[:, 0:nt, :].rearrange("p t (h k) -> p t h k", k=16),
                            axis=mybir.AxisListType.X, op=Alu.add)
                        # a = leaky(score + s_er); e = exp(a)
                        a32 = scr.tile([128, 3 * 4], dt.float32, tag="a32")
                        a3 = a32[:].rearrange("p (t c) -> p t c", c=4)
                        nc.vector.tensor_tensor(out=a3[:, 0:nt, :],
                                                in0=psE3[:, 0:nt, 64:68],
                                                in1=ser3[:, 0:nt, :], op=Alu.add)
                        al = scr.tile([128, 3 * 4], dt.float32, tag="al")
                        al3 = al[:].rearrange("p (t c) -> p t c", c=4)
                        nc.vector.tensor_scalar(out=al3[:, 0:nt, :],
                                                in0=a3[:, 0:nt, :], scalar1=NEG,
                                                scalar2=None, op0=Alu.mult)
                        nc.vector.tensor_tensor(out=al3[:, 0:nt, :],
                                                in0=a3[:, 0:nt, :],
                                                in1=al3[:, 0:nt, :], op=Alu.max)
                        e32 = scr.tile([128, 3 * 4], dt.float32, tag="e32")
                        e3 = e32[:].rearrange("p (t c) -> p t c", c=4)
                        nc.scalar.activation(out=e3[:, 0:nt, :], in_=al3[:, 0:nt, :],
                                             func=Act.Exp, bias=ebias[:])
                        nc.scalar.activation(out=V3[:, 3 * g:3 * g + nt, 128:132],
                                             in_=al3[:, 0:nt, :], func=Act.Exp, bias=ebias[:])
                        # v1 = e * sf
                        nc.vector.tensor_tensor(
                            out=V3[:, 3 * g:3 * g + nt, 0:64]
                                .rearrange("p t (h k) -> p t h k", k=16),
                            in0=psE3[:, 0:nt, 68:132]
                                .rearrange("p t (h k) -> p t h k", k=16),
                            in1=e3[:, 0:nt, :].to_broadcast([128, nt, 4, 16]),
                            op=Alu.mult)
                    # segment sums into psV
                    for t in range(nb2):
                        nc.tensor.matmul(psV[:, :], lhsT=oh3[:, t, :],
                                         rhs=V3[:, t, :],
                                         start=(t == 0), stop=(t == nb2 - 1))
                    # finalize block
                    dn = scr.tile([64, 4], dt.float32, tag="dn")
                    nc.vector.tensor_scalar(out=dn[:], in0=psV[:, 128:132],
                                            scalar1=1e-38, scalar2=None,
                                            op0=Alu.max)
                    rc = scr.tile([64, 4], dt.float32, tag="rc")
                    nc.vector.reciprocal(rc[:], dn[:])
                    nc.vector.tensor_tensor(
                        out=es3[:, b, 0:64].rearrange("p (h k) -> p h k", k=16),
                        in0=psV[:, 0:64].rearrange("p (h k) -> p h k", k=16),
                        in1=rc[:].to_broadcast([64, 4, 16]), op=Alu.mult)
                    nc.vector.tensor_scalar(
                        out=es3[:, b, 64:128],
                        in0=psV[:, 64:128],
                        scalar1=ivd_sb[:, b:b + 1],
                        scalar2=None, op0=Alu.mult)
                    off += Sb

            nc.sync.dma_start(
                esout.rearrange("(t p) c -> p t c", p=64),
                es3)

    nc.compile()
    return nc


_CACHE = {}


def _get_program(caps, STOT, IDXCOLS):
    key = (caps.tobytes(), STOT)
    if key not in _CACHE:
        _CACHE[key] = _build_program(caps, STOT, IDXCOLS)
    return _CACHE[key]


def _install_ntff_shim():
    """The image's antenv lacks axon_hooks; supply it so bass_utils can
    drive NTFF profiling through libaxon_pjrt."""
    import types
    import antenv
    if "antenv.axon_hooks" in sys.modules:
        return
    mod = types.ModuleType("antenv.axon_hooks")
    mod._hook = None
    mod.set_axon_ntff_profile_hook = lambda h: setattr(mod, "_hook", h)
    mod.get_axon_ntff_profile_hook = lambda: mod._hook
    sys.modules["antenv.axon_hooks"] = mod
    antenv.axon_hooks = mod
    from trn_agent_boot.trn_boot import _ntff_profile_via_ctypes
    mod._hook = _ntff_profile_via_ctypes("/opt/axon/libaxon_pjrt.so")


def run(inputs, trace=False, trace_kwargs=None):
    """Build + run; returns (edge_s, out, h) plus the raw BassKernelResults."""
    from concourse.bass_utils import run_bass_kernel_spmd

    caps, STOT, IDXCOLS, per_core_maps = _host_prep(**inputs)
    nc = _get_program(caps, STOT, IDXCOLS)
    in_maps = [{k: np.ascontiguousarray(v) for k, v in m.items()}
               for m in per_core_maps]
    kw = {}
    if trace:
        _install_ntff_shim()
        kw = dict(trace=True, **(trace_kwargs or {}))
    res = run_bass_kernel_spmd(nc, in_maps, core_ids=list(range(NCORES)), **kw)

    edge_s = np.empty((N, DOUT), np.float32)
    out = np.empty((N, DOUT), np.float32)
    h = np.empty((N, DOUT), np.float32)
    for c in range(NCORES):
        r = res.results[c]
        es = np.asarray(r["esout"], np.float32)
        hh = np.asarray(r["hout"], np.float32)
        sl = slice(c * NPC, (c + 1) * NPC)
        out[sl] = es[:NPC, 0:64]
        edge_s[sl] = es[:NPC, 64:128]
        h[sl] = hh[:NPC]
    return (edge_s, out, h), res


def kernel(**inputs):
    (edge_s, out, h), _ = run(inputs)
    return (edge_s, out, h)
